# revision 39
# baseline (speedup 1.0000x reference)
"""Trainium2 Bass kernel for MockGCN segment-reduce problem.

Pipeline (8-way data-parallel, graphs LPT-balanced across cores so every
core has ~identical column counts):
  h1 = relu(x @ W_in + b_in)         [N, 64]
  h2 = relu(h1 @ W_h + b_h)          [N, 64]
  pooled[g] = mean_{i in g} h2[i]    [G, 64]
  out = pooled @ W_out + b_out       [G, 5]

Device layout: features-on-partitions ("T orientation"), nodes 2-packed
across the 128 partitions (64 feats x 2 node streams) and 4-interleaved
along the free axis so a column of the partial tensor holds 4 consecutive
nodes.  The host pads every segment to a multiple of 4 nodes, packs
x into xT_dev [128, C4] float8_e4m3 (halves input DMA traffic; the
~0.9% extra absmax error is well inside the 2e-2 gate), and the device
emits per-2-node relu'd h2 pairs [128, 2*C4] bf16.  The host folds the
two node streams, does the per-segment cumsum combine, mean division,
pad correction, and the tiny [G,64]@[64,5] matmul.

Stage map (per 512-col quantum = 2048 nodes):
  DMA in xT chunk [128, 1024] fp8 (2 quanta per dma_start; biases ride
      the SWDGE ring in parallel with x0/w1/w2 on HWDGE at startup)
  PE:  warm-up dummy matmuls first (p-state ramps to 2.4GHz before real
       work), then MM-A/MM-B (row-tiled, fp8 x bf16) -> psum ab [128,1024]
  D1:  relu(ab + b1) -> rhs2 [128,1024] bf16 SBUF  (ScalarE, 1038ns)
  PE:  MM x2 (K=128, blkdiag(W2,W2)) -> psum cd [128,1024]
  D2:  relu(cd + b2) -> h2r [128,1024] bf16 SBUF   (VectorE 1192ns;
       ScalarE steals every 14th quantum plus tail quanta to balance)
  DMA out h2r -> pout[:, 2qQ:2(q+1)Q] (2-quantum batches; per-quantum
      flushes for the last 2 quanta shorten the end chain)

The PSUM->SBUF drains (D1/D2) are the hard bottleneck: 32M psum fp32
elements/core can only pass through ScalarE (0.833ns/col + 185ns/instr)
and VectorE (1.042ns/col + 125ns/instr) - Pool/gpsimd and DMA cannot
read PSUM, PSUM's 8 banks cap drain instruction size at 1024 cols, and
DVE 2x modes require 2-byte operands (PSUM is fp32-only on TRN2).
Balanced floor ~1109ns/quantum; this schedule reaches ~1190 steady +
~4.5us startup + ~4us tail = 152.1us (vs 158.3us for the previous
device-side-fold version).
"""

import sys

if "/opt/trn_rl_repo" not in sys.path:
    sys.path.insert(0, "/opt/trn_rl_repo")

from contextlib import ExitStack

import ml_dtypes
import numpy as np

N_CORES = 8
G_TOTAL = 8192
F_IN = 32
H_DIM = 64
Q_COLS = 512  # partial columns per quantum
PACK = 4  # nodes per partial column
DMA_BATCH = 2  # quanta per input DMA transfer
OUT_BATCH = 2  # quanta per output DMA transfer
D2_ACT_EVERY = 14  # ScalarE steals D2 from VectorE every k-th quantum
DEVICE_R = False  # sum the two node streams on device vs on host
DELAY_STEAL = False  # emit the ScalarE-stolen D2 one quantum late
R_ON_POOL = True  # device stream-sum engine: GpSimd (True) or VectorE (False)
X_FP8 = True  # stream x as float8_e4m3 (halves input DMA traffic)
WEIGHTS_ON_SYNC = True  # weights ride HWDGE behind the first x chunk
TAIL_WINDOW = 4  # in the last K quanta, ScalarE takes extra D2 steals
TAIL_EVERY = 2  # ...every k-th quantum of the tail window
PE_WARM_MM = 4  # dummy matmuls to ramp the PE p-state before real work
LAST_R_ON_DVE = True  # final quantum's stream-fold on VectorE (shorter tail)
HALF_STEAL = False  # steal half a D2 every D2_ACT_EVERY//2 quanta instead
TAIL_FLUSH = 2  # flush output per-quantum for the last K quanta

_BUILD_CACHE: dict = {}
_LAST_IN_MAPS: list | None = None


def _build_program(c4: int):
    """Build + compile the 8-core SPMD Bass program for C4 partial columns."""
    import concourse.tile as tile
    from concourse import bacc, mybir

    f32 = mybir.dt.float32
    bf16 = mybir.dt.bfloat16
    xdt = mybir.dt.float8e4 if X_FP8 else bf16
    Relu = mybir.ActivationFunctionType.Relu
    add_op = mybir.AluOpType.add
    max_op = mybir.AluOpType.max

    nq = c4 // Q_COLS
    assert c4 % Q_COLS == 0

    nc = bacc.Bacc(
        "TRN2",
        target_bir_lowering=False,
        debug=False,
        enable_asserts=False,
        num_devices=N_CORES,
    )

    xT = nc.dram_tensor("xT", [128, c4], xdt, kind="ExternalInput").ap()
    w1 = nc.dram_tensor("w1", [128, 128], bf16, kind="ExternalInput").ap()
    w2 = nc.dram_tensor("w2", [128, 128], bf16, kind="ExternalInput").ap()
    b1 = nc.dram_tensor("b1", [128, 1], f32, kind="ExternalInput").ap()
    b2 = nc.dram_tensor("b2", [128, 1], f32, kind="ExternalInput").ap()
    pw = c4 if DEVICE_R else 2 * c4
    pout = nc.dram_tensor("pout", [128, pw], bf16, kind="ExternalOutput").ap()

    with ExitStack() as ctx:
        tc = ctx.enter_context(tile.TileContext(nc))
        singles = ctx.enter_context(tc.tile_pool(name="singles", bufs=1))
        xpool = ctx.enter_context(tc.tile_pool(name="xc", bufs=3))
        rpool = ctx.enter_context(tc.tile_pool(name="rhs2", bufs=3))
        hpool = ctx.enter_context(tc.tile_pool(name="h2r", bufs=3))
        ppool = ctx.enter_context(tc.tile_pool(name="pc", bufs=3))
        abpool = ctx.enter_context(tc.tile_pool(name="ab", bufs=2, space="PSUM"))
        cdpool = ctx.enter_context(tc.tile_pool(name="cd", bufs=2, space="PSUM"))

        w1sb = singles.tile([128, 128], bf16)
        w2sb = singles.tile([128, 128], bf16)
        b1sb = singles.tile([128, 1], f32)
        b2sb = singles.tile([128, 1], f32)
        Q = Q_COLS
        xc0 = None
        if WEIGHTS_ON_SYNC:
            # Biases ride the (otherwise idle) SWDGE ring in parallel; the
            # first x chunk leads the HWDGE FIFO with w1/w2 right behind it.
            nc.gpsimd.dma_start(out=b1sb, in_=b1)
            nc.gpsimd.dma_start(out=b2sb, in_=b2)
            # Chunk 0 is split so quantum 0's columns land in one minimal
            # transfer; the rest of the chunk follows behind the weights.
            take0 = min(DMA_BATCH, nq)
            xc0 = xpool.tile([128, DMA_BATCH * Q], xdt, name="xc")
            nc.sync.dma_start(out=xc0[:, 0:Q], in_=xT[:, 0:Q])
            nc.sync.dma_start(out=w1sb, in_=w1)
            if take0 > 1:
                nc.sync.dma_start(
                    out=xc0[:, Q : take0 * Q], in_=xT[:, Q : take0 * Q]
                )
            nc.sync.dma_start(out=w2sb, in_=w2)
        else:
            nc.gpsimd.dma_start(out=w1sb, in_=w1)
            nc.gpsimd.dma_start(out=w2sb, in_=w2)
            nc.gpsimd.dma_start(out=b1sb, in_=b1)
            nc.gpsimd.dma_start(out=b2sb, in_=b2)

        # Pre-warm the ScalarE activation table (~2.7us PSEUDO_LOAD_ACT_FUNC_SET
        # attaches to the first ACTIVATE) so it overlaps the first x-chunk DMA.
        warm = singles.tile([128, 1], f32)
        nc.vector.memset(warm, 0.0)
        nc.scalar.activation(warm, warm, Relu)

        # Pre-warm the PE p-state: dummy matmuls on a zeroed scratch tile keep
        # the PE continuously busy while the first x chunk is in flight, so
        # real matmuls start at the full 2.4GHz clock instead of 1.2GHz.
        ab0 = None
        if PE_WARM_MM:
            scratch = singles.tile([128, 512], bf16)
            nc.vector.memset(scratch, 0.0)
            ab0 = abpool.tile([128, 2 * Q_COLS], f32, name="ab")
            for _ in range(PE_WARM_MM):
                nc.tensor.matmul(
                    out=ab0[:, 0:Q_COLS],
                    lhsT=scratch[:, 0:128],
                    rhs=scratch,
                    start=True,
                    stop=True,
                )

        # Software-pipelined PE stream: stage-1 of quantum q+1 is emitted
        # BEFORE stage-2 of quantum q, so the in-order PE queue never blocks
        # the next D1's input behind a stage-2 that waits on the current D1.
        # This breaks the D1->stage2->stage1->D1 dependency cycle (~1209ns)
        # that otherwise paces the steady state above the engine-busy floor.
        xc = None
        h2r = None
        pc = None
        pending = None  # delayed ScalarE-stolen D2: (cd, dst, pc, h2r, q)

        def fetch_chunk(q):
            take = min(DMA_BATCH, nq - q)
            t = xpool.tile([128, DMA_BATCH * Q], xdt, name="xc")
            nc.sync.dma_start(
                out=t[:, 0 : take * Q], in_=xT[:, q * Q : (q + take) * Q]
            )
            return t

        def stage1(q, xc_t, ab_t):
            jx = (q % DMA_BATCH) * Q
            nc.tensor.matmul(
                out=ab_t[:, 0:Q],
                lhsT=w1sb[0:64, :],
                rhs=xc_t[0:64, jx : jx + Q],
                start=True,
                stop=True,
            )
            nc.tensor.matmul(
                out=ab_t[:, Q : 2 * Q],
                lhsT=w1sb[64:128, :],
                rhs=xc_t[64:128, jx : jx + Q],
                start=True,
                stop=True,
            )

        # Prologue: quantum 0's stage-1 runs right after the PE warm-up.
        xc = xc0 if xc0 is not None else fetch_chunk(0)
        ab = ab0 if ab0 is not None else abpool.tile([128, 2 * Q], f32, name="ab")
        stage1(0, xc, ab)

        for q in range(nq):
            if q % OUT_BATCH == 0:
                h2r = hpool.tile([128, OUT_BATCH * 2 * Q], bf16)
            jh = (q % OUT_BATCH) * 2 * Q

            # Prefetch + stage-1 for quantum q+1 (ahead of this quantum's
            # stage-2 in the PE queue).
            if q + 1 < nq:
                if (q + 1) % DMA_BATCH == 0:
                    xc_n = fetch_chunk(q + 1)
                else:
                    xc_n = xc
                ab_n = abpool.tile([128, 2 * Q], f32, name="ab")
                stage1(q + 1, xc_n, ab_n)
            else:
                xc_n = None
                ab_n = None

            rhs2 = rpool.tile([128, 2 * Q], bf16)
            nc.scalar.activation(rhs2, ab, Relu, bias=b1sb)

            def finish_quantum(qq, dst_t, pc_t, h2r_t, fold_on_dve=False):
                if DEVICE_R:
                    r_eng = (
                        nc.vector
                        if (
                            fold_on_dve
                            or not R_ON_POOL
                            or (LAST_R_ON_DVE and qq == nq - 1)
                        )
                        else nc.gpsimd
                    )
                    r_eng.tensor_add(
                        pc_t[:, (qq % OUT_BATCH) * Q : (qq % OUT_BATCH + 1) * Q],
                        dst_t[:, 0:Q],
                        dst_t[:, Q : 2 * Q],
                    )
                tfs = ((nq - TAIL_FLUSH) // OUT_BATCH) * OUT_BATCH  # batch-aligned
                if TAIL_FLUSH > 0 and qq >= tfs:
                    # Tail: flush each quantum's output as soon as it lands,
                    # so the final DMA chain is short.
                    j = qq % OUT_BATCH
                    if DEVICE_R:
                        nc.sync.dma_start(
                            out=pout[:, qq * Q : (qq + 1) * Q],
                            in_=pc_t[:, j * Q : (j + 1) * Q],
                        )
                    else:
                        nc.sync.dma_start(
                            out=pout[:, 2 * qq * Q : 2 * (qq + 1) * Q],
                            in_=h2r_t[:, j * 2 * Q : (j + 1) * 2 * Q],
                        )
                elif qq % OUT_BATCH == OUT_BATCH - 1 or qq == nq - 1:
                    nb = qq % OUT_BATCH + 1  # quanta in this (possibly short) batch
                    q0 = qq - (nb - 1)
                    if DEVICE_R:
                        nc.sync.dma_start(
                            out=pout[:, q0 * Q : (qq + 1) * Q],
                            in_=pc_t[:, 0 : nb * Q],
                        )
                    else:
                        nc.sync.dma_start(
                            out=pout[:, 2 * q0 * Q : 2 * (qq + 1) * Q],
                            in_=h2r_t[:, 0 : nb * 2 * Q],
                        )

            # Flush a D2 stolen by ScalarE one quantum ago (its deps are long
            # ready, so ScalarE never stalls on this quantum's stage-2).
            if pending is not None:
                cd_p, dst_p, pc_p, h2r_p, q_p = pending
                nc.scalar.activation(dst_p, cd_p, Relu, bias=b2sb)
                finish_quantum(q_p, dst_p, pc_p, h2r_p)
                pending = None

            cd = cdpool.tile([128, 2 * Q], f32)
            nc.tensor.matmul(
                out=cd[:, 0:Q], lhsT=w2sb, rhs=rhs2[:, 0:Q], start=True, stop=True
            )
            nc.tensor.matmul(
                out=cd[:, Q : 2 * Q],
                lhsT=w2sb,
                rhs=rhs2[:, Q : 2 * Q],
                start=True,
                stop=True,
            )

            if DEVICE_R and q % OUT_BATCH == 0:
                pc = ppool.tile([128, OUT_BATCH * Q], bf16)
            dst = h2r[:, jh : jh + 2 * Q]
            steady_steal = (
                q % D2_ACT_EVERY == D2_ACT_EVERY - 1
                and q % OUT_BATCH == OUT_BATCH - 1
                and q < nq - 1
            )
            half_steal = (
                HALF_STEAL
                and q % (D2_ACT_EVERY // 2) == D2_ACT_EVERY // 2 - 1
                and q < nq - 1
            )
            tail_steal = (
                q >= nq - TAIL_WINDOW and q % TAIL_EVERY == TAIL_EVERY - 1
            )
            if HALF_STEAL and not tail_steal:
                if half_steal:
                    nc.scalar.activation(
                        dst[:, 0:Q], cd[:, 0:Q], Relu, bias=b2sb
                    )
                    nc.vector.tensor_scalar(
                        dst[:, Q : 2 * Q],
                        cd[:, Q : 2 * Q],
                        b2sb,
                        0.0,
                        add_op,
                        max_op,
                    )
                else:
                    nc.vector.tensor_scalar(dst, cd, b2sb, 0.0, add_op, max_op)
                finish_quantum(q, dst, pc, h2r)
            elif steady_steal or tail_steal:
                if DELAY_STEAL:
                    pending = (cd, dst, pc, h2r, q)
                else:
                    nc.scalar.activation(dst, cd, Relu, bias=b2sb)
                    finish_quantum(q, dst, pc, h2r)
            else:
                nc.vector.tensor_scalar(dst, cd, b2sb, 0.0, add_op, max_op)
                finish_quantum(q, dst, pc, h2r)

            xc = xc_n
            ab = ab_n

    nc.compile()
    return nc


def _get_program(c4: int):
    if c4 not in _BUILD_CACHE:
        _BUILD_CACHE[c4] = _build_program(c4)
    return _BUILD_CACHE[c4]


def kernel(x, batch, num_graphs, W_in, b_in, W_h, b_h, W_out, b_out):
    from concourse import bass_utils

    x = np.asarray(x, dtype=np.float32)
    batch = np.asarray(batch).astype(np.int64)
    g_total = int(num_graphs)
    W_in = np.asarray(W_in, dtype=np.float32)
    b_in = np.asarray(b_in, dtype=np.float32)
    W_h = np.asarray(W_h, dtype=np.float32)
    b_h = np.asarray(b_h, dtype=np.float32)
    W_out = np.asarray(W_out, dtype=np.float32)
    b_out = np.asarray(b_out, dtype=np.float32)

    if batch.size and np.any(np.diff(batch) < 0):
        order = np.argsort(batch, kind="stable")
        x = x[order]
        batch = batch[order]

    n_nodes, f_in = x.shape
    h_dim = W_in.shape[1]
    assert f_in == F_IN and h_dim == H_DIM

    counts = np.bincount(batch, minlength=g_total).astype(np.int64)
    node_starts = np.concatenate([[0], np.cumsum(counts)])  # [G+1]

    # Per-graph padded counts (multiple of PACK).
    pc_counts = (counts + PACK - 1) // PACK * PACK

    # Balanced graph->core assignment (LPT): the SPMD program runs every core
    # for the max core's column count, so minimizing the max load saves whole
    # quanta versus contiguous 1024-graph shards (~0.6% spread).
    import heapq

    order_g = np.argsort(pc_counts, kind="stable")[::-1]
    heap = [(0, c) for c in range(N_CORES)]
    heapq.heapify(heap)
    core_graphs = [[] for _ in range(N_CORES)]
    loads = [0] * N_CORES
    for g in order_g:
        load, c = heapq.heappop(heap)
        core_graphs[c].append(g)
        load += int(pc_counts[g])
        loads[c] = load
        heapq.heappush(heap, (load, c))
    core_graphs = [np.asarray(gs, dtype=np.int64) for gs in core_graphs]
    c4_per_core = [t // PACK for t in loads]
    align = Q_COLS  # partial DMA batches are supported, so quantum-aligned
    c4 = max(c4_per_core)
    c4 = (c4 + align - 1) // align * align

    # Constant tensors shared by all cores.
    w1blk = np.zeros((128, 128), dtype=np.float32)
    w1blk[0:32, 0:64] = W_in
    w1blk[32:64, 64:128] = W_in
    w1blk[64:96, 0:64] = W_in
    w1blk[96:128, 64:128] = W_in
    w1blk = w1blk.astype(ml_dtypes.bfloat16)
    w2blk = np.zeros((128, 128), dtype=np.float32)
    w2blk[0:64, 0:64] = W_h
    w2blk[64:128, 64:128] = W_h
    w2blk = w2blk.astype(ml_dtypes.bfloat16)
    b1cat = np.tile(b_in, 2).reshape(128, 1).astype(np.float32)
    b2cat = np.tile(b_h, 2).reshape(128, 1).astype(np.float32)

    # Per-core packed inputs.
    in_maps = []
    for c in range(N_CORES):
        gc = core_graphs[c]
        cnt_c = counts[gc]
        pc_c = pc_counts[gc]
        pad_starts = np.concatenate([[0], np.cumsum(pc_c)])  # [len(gc)+1]

        x_padded = np.zeros((c4 * PACK, f_in), dtype=np.float32)
        tot = int(cnt_c.sum())
        if tot > 0:
            # Gather this core's nodes graph-by-graph into padded slots.
            within = np.arange(tot) - np.repeat(
                np.concatenate([[0], np.cumsum(cnt_c)])[:-1], cnt_c
            )
            src = np.repeat(node_starts[gc], cnt_c) + within
            dst = np.repeat(pad_starts[:-1], cnt_c) + within
            x_padded[dst] = x[src]
        xT_dev = (
            x_padded.reshape(c4, PACK, f_in).transpose(1, 2, 0).reshape(128, c4)
        )
        xdt_np = ml_dtypes.float8_e4m3 if X_FP8 else ml_dtypes.bfloat16
        xT_dev = np.ascontiguousarray(xT_dev).astype(xdt_np)
        in_maps.append(
            {
                "xT": xT_dev,
                "w1": w1blk,
                "w2": w2blk,
                "b1": b1cat,
                "b2": b2cat,
            }
        )

    global _LAST_IN_MAPS
    _LAST_IN_MAPS = in_maps

    nc = _get_program(c4)
    res = bass_utils.run_bass_kernel_spmd(
        nc, in_maps, core_ids=list(range(N_CORES))
    )

    # Pad-node contribution, exactly as the device computes it for x=0 rows:
    # h1raw = 0 (fp32 psum) -> D1: bf16(relu(b1)) -> S2 bf16 matmul (fp32 acc)
    # -> D2: bf16(relu(. + b2)).
    bf = ml_dtypes.bfloat16
    h1p = np.maximum(b_in, 0.0).astype(bf).astype(np.float32)
    w2bf = W_h.astype(bf).astype(np.float32)
    vpad = np.maximum(h1p @ w2bf + b_h, 0.0).astype(bf).astype(np.float32)
    vpad_bf = vpad

    out = np.zeros((g_total, W_out.shape[1]), dtype=np.float32)
    for c in range(N_CORES):
        gc = core_graphs[c]
        cnt_c = counts[gc].astype(np.float64)
        pc_c = pc_counts[gc]
        pad_starts = np.concatenate([[0], np.cumsum(pc_c)])
        col_starts = pad_starts // PACK  # [len(gc)+1]

        nq = c4 // Q_COLS
        P = np.asarray(res.results[c]["pout"]).astype(np.float32)
        if DEVICE_R:
            R1 = P[0:64, :] + P[64:128, :]  # [64, c4]
        else:
            Pr = P.reshape(128, nq, 2, Q_COLS)
            # fold partition halves (node pairs) and streams
            R1 = (Pr[0:64] + Pr[64:128]).sum(axis=2).reshape(64, c4)
        cs = np.concatenate(
            [np.zeros((64, 1)), np.cumsum(R1.astype(np.float64), axis=1)], axis=1
        )  # [64, c4+1]
        seg_sum = (cs[:, col_starts[1:]] - cs[:, col_starts[:-1]]).T  # [g, 64]

        n_pad = (pc_c - counts[gc]).astype(np.float64)
        seg_sum = seg_sum - n_pad[:, None] * vpad_bf[None, :].astype(np.float64)
        denom = np.maximum(cnt_c, 1.0)
        mean = seg_sum / denom[:, None]
        mean[cnt_c == 0] = 0.0
        out[gc] = mean.astype(np.float32) @ W_out + b_out

    return out



# revision 45
# speedup vs baseline: 1.0005x; 1.0005x over previous
"""Trainium2 Bass kernel for MockGCN segment-reduce problem.

Pipeline (8-way data-parallel, graphs LPT-balanced across cores so every
core has ~identical column counts):
  h1 = relu(x @ W_in + b_in)         [N, 64]
  h2 = relu(h1 @ W_h + b_h)          [N, 64]
  pooled[g] = mean_{i in g} h2[i]    [G, 64]
  out = pooled @ W_out + b_out       [G, 5]

Device layout: features-on-partitions ("T orientation"), nodes 2-packed
across the 128 partitions (64 feats x 2 node streams) and 4-interleaved
along the free axis so a column of the partial tensor holds 4 consecutive
nodes.  The host pads every segment to a multiple of 4 nodes, packs
x into xT_dev [128, C4] float8_e4m3 (halves input DMA traffic; the
~0.9% extra absmax error is well inside the 2e-2 gate), and the device
emits per-2-node relu'd h2 pairs [128, 2*C4] bf16.  The host folds the
two node streams, does the per-segment cumsum combine, mean division,
pad correction, and the tiny [G,64]@[64,5] matmul.

Stage map (per 512-col quantum = 2048 nodes):
  DMA in xT chunk [128, 1024] fp8 (2 quanta per dma_start; biases ride
      the SWDGE ring in parallel with x0/w1/w2 on HWDGE at startup)
  PE:  warm-up dummy matmuls first (p-state ramps to 2.4GHz before real
       work), then MM-A/MM-B (row-tiled, fp8 x bf16) -> psum ab [128,1024]
  D1:  relu(ab + b1) -> rhs2 [128,1024] bf16 SBUF  (ScalarE, 1038ns)
  PE:  MM x2 (K=128, blkdiag(W2,W2)) -> psum cd [128,1024]
  D2:  relu(cd + b2) -> h2r [128,1024] bf16 SBUF   (VectorE 1192ns;
       ScalarE steals every 14th quantum plus tail quanta to balance)
  DMA out h2r -> pout[:, 2qQ:2(q+1)Q] (2-quantum batches; per-quantum
      flushes for the last 2 quanta shorten the end chain)

The PSUM->SBUF drains (D1/D2) are the hard bottleneck: 32M psum fp32
elements/core can only pass through ScalarE (0.833ns/col + 185ns/instr)
and VectorE (1.042ns/col + 125ns/instr) - Pool/gpsimd and DMA cannot
read PSUM, PSUM's 8 banks cap drain instruction size at 1024 cols, and
DVE 2x modes require 2-byte operands (PSUM is fp32-only on TRN2).
Balanced floor ~1109ns/quantum; this schedule reaches ~1190 steady +
~4.4us startup + ~4us tail = 152.0us (vs 158.3us for the previous
device-side-fold version).
"""

import sys

if "/opt/trn_rl_repo" not in sys.path:
    sys.path.insert(0, "/opt/trn_rl_repo")

from contextlib import ExitStack

import ml_dtypes
import numpy as np

N_CORES = 8
G_TOTAL = 8192
F_IN = 32
H_DIM = 64
Q_COLS = 512  # partial columns per quantum
PACK = 4  # nodes per partial column
DMA_BATCH = 2  # quanta per input DMA transfer
OUT_BATCH = 2  # quanta per output DMA transfer
D2_ACT_EVERY = 14  # ScalarE steals D2 from VectorE every k-th quantum
DEVICE_R = False  # sum the two node streams on device vs on host
DELAY_STEAL = False  # emit the ScalarE-stolen D2 one quantum late
R_ON_POOL = True  # device stream-sum engine: GpSimd (True) or VectorE (False)
X_FP8 = True  # stream x as float8_e4m3 (halves input DMA traffic)
WEIGHTS_ON_SYNC = True  # weights ride HWDGE behind the first x chunk
TAIL_WINDOW = 4  # in the last K quanta, ScalarE takes extra D2 steals
TAIL_EVERY = 2  # ...every k-th quantum of the tail window
PE_WARM_MM = 4  # dummy matmuls to ramp the PE p-state before real work
LAST_R_ON_DVE = True  # final quantum's stream-fold on VectorE (shorter tail)
HALF_STEAL = False  # steal half a D2 every D2_ACT_EVERY//2 quanta instead
TAIL_FLUSH = 2  # flush output per-quantum for the last K quanta
RAMP_SPLIT_D1 = 0  # split the first K quanta's D1 across both drain engines
LAST_D2_SPLIT = False  # split the final D2 + flush per half (shorter end chain)

_BUILD_CACHE: dict = {}
_LAST_IN_MAPS: list | None = None


def _build_program(c4: int):
    """Build + compile the 8-core SPMD Bass program for C4 partial columns."""
    import concourse.tile as tile
    from concourse import bacc, mybir

    f32 = mybir.dt.float32
    bf16 = mybir.dt.bfloat16
    xdt = mybir.dt.float8e4 if X_FP8 else bf16
    Relu = mybir.ActivationFunctionType.Relu
    add_op = mybir.AluOpType.add
    max_op = mybir.AluOpType.max

    nq = c4 // Q_COLS
    assert c4 % Q_COLS == 0

    nc = bacc.Bacc(
        "TRN2",
        target_bir_lowering=False,
        debug=False,
        enable_asserts=False,
        num_devices=N_CORES,
    )

    xT = nc.dram_tensor("xT", [128, c4], xdt, kind="ExternalInput").ap()
    w1 = nc.dram_tensor("w1", [128, 128], bf16, kind="ExternalInput").ap()
    w2 = nc.dram_tensor("w2", [128, 128], bf16, kind="ExternalInput").ap()
    b1 = nc.dram_tensor("b1", [128, 1], f32, kind="ExternalInput").ap()
    b2 = nc.dram_tensor("b2", [128, 1], f32, kind="ExternalInput").ap()
    pw = c4 if DEVICE_R else 2 * c4
    pout = nc.dram_tensor("pout", [128, pw], bf16, kind="ExternalOutput").ap()

    with ExitStack() as ctx:
        tc = ctx.enter_context(tile.TileContext(nc))
        singles = ctx.enter_context(tc.tile_pool(name="singles", bufs=1))
        xpool = ctx.enter_context(tc.tile_pool(name="xc", bufs=3))
        rpool = ctx.enter_context(tc.tile_pool(name="rhs2", bufs=3))
        hpool = ctx.enter_context(tc.tile_pool(name="h2r", bufs=3))
        ppool = ctx.enter_context(tc.tile_pool(name="pc", bufs=3))
        abpool = ctx.enter_context(tc.tile_pool(name="ab", bufs=2, space="PSUM"))
        cdpool = ctx.enter_context(tc.tile_pool(name="cd", bufs=2, space="PSUM"))

        w1sb = singles.tile([128, 128], bf16)
        w2sb = singles.tile([128, 128], bf16)
        b1sb = singles.tile([128, 1], f32)
        b2sb = singles.tile([128, 1], f32)
        Q = Q_COLS
        xc0 = None
        if WEIGHTS_ON_SYNC:
            # Biases ride the (otherwise idle) SWDGE ring in parallel; the
            # first x chunk leads the HWDGE FIFO with w1/w2 right behind it.
            nc.gpsimd.dma_start(out=w1sb, in_=w1)
            nc.gpsimd.dma_start(out=b1sb, in_=b1)
            nc.gpsimd.dma_start(out=b2sb, in_=b2)
            # Chunk 0 is split so quantum 0's columns land in one minimal
            # transfer; the rest of the chunk follows behind the weights.
            take0 = min(DMA_BATCH, nq)
            xc0 = xpool.tile([128, DMA_BATCH * Q], xdt, name="xc")
            nc.sync.dma_start(out=xc0[:, 0:Q], in_=xT[:, 0:Q])
            if take0 > 1:
                nc.sync.dma_start(
                    out=xc0[:, Q : take0 * Q], in_=xT[:, Q : take0 * Q]
                )
            nc.sync.dma_start(out=w2sb, in_=w2)
        else:
            nc.gpsimd.dma_start(out=w1sb, in_=w1)
            nc.gpsimd.dma_start(out=w2sb, in_=w2)
            nc.gpsimd.dma_start(out=b1sb, in_=b1)
            nc.gpsimd.dma_start(out=b2sb, in_=b2)

        # Pre-warm the ScalarE activation table (~2.7us PSEUDO_LOAD_ACT_FUNC_SET
        # attaches to the first ACTIVATE) so it overlaps the first x-chunk DMA.
        warm = singles.tile([128, 1], f32)
        nc.vector.memset(warm, 0.0)
        nc.scalar.activation(warm, warm, Relu)

        # Pre-warm the PE p-state: dummy matmuls on a zeroed scratch tile keep
        # the PE continuously busy while the first x chunk is in flight, so
        # real matmuls start at the full 2.4GHz clock instead of 1.2GHz.
        ab0 = None
        if PE_WARM_MM:
            scratch = singles.tile([128, 512], bf16)
            nc.vector.memset(scratch, 0.0)
            ab0 = abpool.tile([128, 2 * Q_COLS], f32, name="ab")
            for _ in range(PE_WARM_MM):
                nc.tensor.matmul(
                    out=ab0[:, 0:Q_COLS],
                    lhsT=scratch[:, 0:128],
                    rhs=scratch,
                    start=True,
                    stop=True,
                )

        # Software-pipelined PE stream: stage-1 of quantum q+1 is emitted
        # BEFORE stage-2 of quantum q, so the in-order PE queue never blocks
        # the next D1's input behind a stage-2 that waits on the current D1.
        # This breaks the D1->stage2->stage1->D1 dependency cycle (~1209ns)
        # that otherwise paces the steady state above the engine-busy floor.
        xc = None
        h2r = None
        pc = None
        pending = None  # delayed ScalarE-stolen D2: (cd, dst, pc, h2r, q)

        def fetch_chunk(q):
            take = min(DMA_BATCH, nq - q)
            t = xpool.tile([128, DMA_BATCH * Q], xdt, name="xc")
            nc.sync.dma_start(
                out=t[:, 0 : take * Q], in_=xT[:, q * Q : (q + take) * Q]
            )
            return t

        def stage1(q, xc_t, ab_t):
            jx = (q % DMA_BATCH) * Q
            nc.tensor.matmul(
                out=ab_t[:, 0:Q],
                lhsT=w1sb[0:64, :],
                rhs=xc_t[0:64, jx : jx + Q],
                start=True,
                stop=True,
            )
            nc.tensor.matmul(
                out=ab_t[:, Q : 2 * Q],
                lhsT=w1sb[64:128, :],
                rhs=xc_t[64:128, jx : jx + Q],
                start=True,
                stop=True,
            )

        # Prologue: quantum 0's stage-1 runs right after the PE warm-up.
        xc = xc0 if xc0 is not None else fetch_chunk(0)
        ab = ab0 if ab0 is not None else abpool.tile([128, 2 * Q], f32, name="ab")
        stage1(0, xc, ab)

        for q in range(nq):
            if q % OUT_BATCH == 0:
                h2r = hpool.tile([128, OUT_BATCH * 2 * Q], bf16)
            jh = (q % OUT_BATCH) * 2 * Q

            # Prefetch + stage-1 for quantum q+1 (ahead of this quantum's
            # stage-2 in the PE queue).
            if q + 1 < nq:
                if (q + 1) % DMA_BATCH == 0:
                    xc_n = fetch_chunk(q + 1)
                else:
                    xc_n = xc
                ab_n = abpool.tile([128, 2 * Q], f32, name="ab")
                stage1(q + 1, xc_n, ab_n)
            else:
                xc_n = None
                ab_n = None

            rhs2 = rpool.tile([128, 2 * Q], bf16)
            if q < RAMP_SPLIT_D1:
                # Pipeline fill: VectorE is idle until its first D2 arrives,
                # so splitting the first D1s across both engines halves the
                # serial latency of the pipe-fill chain.
                nc.scalar.activation(
                    rhs2[:, 0:Q], ab[:, 0:Q], Relu, bias=b1sb
                )
                nc.vector.tensor_scalar(
                    rhs2[:, Q : 2 * Q],
                    ab[:, Q : 2 * Q],
                    b1sb,
                    0.0,
                    add_op,
                    max_op,
                )
            else:
                nc.scalar.activation(rhs2, ab, Relu, bias=b1sb)

            def finish_quantum(qq, dst_t, pc_t, h2r_t, fold_on_dve=False):
                if DEVICE_R:
                    r_eng = (
                        nc.vector
                        if (
                            fold_on_dve
                            or not R_ON_POOL
                            or (LAST_R_ON_DVE and qq == nq - 1)
                        )
                        else nc.gpsimd
                    )
                    r_eng.tensor_add(
                        pc_t[:, (qq % OUT_BATCH) * Q : (qq % OUT_BATCH + 1) * Q],
                        dst_t[:, 0:Q],
                        dst_t[:, Q : 2 * Q],
                    )
                tfs = ((nq - TAIL_FLUSH) // OUT_BATCH) * OUT_BATCH  # batch-aligned
                if TAIL_FLUSH > 0 and qq >= tfs:
                    # Tail: flush each quantum's output as soon as it lands,
                    # so the final DMA chain is short.
                    j = qq % OUT_BATCH
                    if DEVICE_R:
                        nc.sync.dma_start(
                            out=pout[:, qq * Q : (qq + 1) * Q],
                            in_=pc_t[:, j * Q : (j + 1) * Q],
                        )
                    else:
                        nc.sync.dma_start(
                            out=pout[:, 2 * qq * Q : 2 * (qq + 1) * Q],
                            in_=h2r_t[:, j * 2 * Q : (j + 1) * 2 * Q],
                        )
                elif qq % OUT_BATCH == OUT_BATCH - 1 or qq == nq - 1:
                    nb = qq % OUT_BATCH + 1  # quanta in this (possibly short) batch
                    q0 = qq - (nb - 1)
                    if DEVICE_R:
                        nc.sync.dma_start(
                            out=pout[:, q0 * Q : (qq + 1) * Q],
                            in_=pc_t[:, 0 : nb * Q],
                        )
                    else:
                        nc.sync.dma_start(
                            out=pout[:, 2 * q0 * Q : 2 * (qq + 1) * Q],
                            in_=h2r_t[:, 0 : nb * 2 * Q],
                        )

            # Flush a D2 stolen by ScalarE one quantum ago (its deps are long
            # ready, so ScalarE never stalls on this quantum's stage-2).
            if pending is not None:
                cd_p, dst_p, pc_p, h2r_p, q_p = pending
                nc.scalar.activation(dst_p, cd_p, Relu, bias=b2sb)
                finish_quantum(q_p, dst_p, pc_p, h2r_p)
                pending = None

            cd = cdpool.tile([128, 2 * Q], f32)
            nc.tensor.matmul(
                out=cd[:, 0:Q], lhsT=w2sb, rhs=rhs2[:, 0:Q], start=True, stop=True
            )
            nc.tensor.matmul(
                out=cd[:, Q : 2 * Q],
                lhsT=w2sb,
                rhs=rhs2[:, Q : 2 * Q],
                start=True,
                stop=True,
            )

            if DEVICE_R and q % OUT_BATCH == 0:
                pc = ppool.tile([128, OUT_BATCH * Q], bf16)
            dst = h2r[:, jh : jh + 2 * Q]
            steady_steal = (
                q % D2_ACT_EVERY == D2_ACT_EVERY - 1
                and q % OUT_BATCH == OUT_BATCH - 1
                and q < nq - 1
            )
            half_steal = (
                HALF_STEAL
                and q % (D2_ACT_EVERY // 2) == D2_ACT_EVERY // 2 - 1
                and q < nq - 1
            )
            tail_steal = (
                q >= nq - TAIL_WINDOW and q % TAIL_EVERY == TAIL_EVERY - 1
            )
            if HALF_STEAL and not tail_steal:
                if half_steal:
                    nc.scalar.activation(
                        dst[:, 0:Q], cd[:, 0:Q], Relu, bias=b2sb
                    )
                    nc.vector.tensor_scalar(
                        dst[:, Q : 2 * Q],
                        cd[:, Q : 2 * Q],
                        b2sb,
                        0.0,
                        add_op,
                        max_op,
                    )
                else:
                    nc.vector.tensor_scalar(dst, cd, b2sb, 0.0, add_op, max_op)
                finish_quantum(q, dst, pc, h2r)
            elif steady_steal or tail_steal:
                if DELAY_STEAL:
                    pending = (cd, dst, pc, h2r, q)
                else:
                    nc.scalar.activation(dst, cd, Relu, bias=b2sb)
                    finish_quantum(q, dst, pc, h2r)
            elif LAST_D2_SPLIT and q == nq - 1 and not DEVICE_R:
                # Final quantum: drain + flush in halves so the first half's
                # output DMA overlaps the second half's drain.
                nc.vector.tensor_scalar(
                    dst[:, 0:Q], cd[:, 0:Q], b2sb, 0.0, add_op, max_op
                )
                nc.sync.dma_start(
                    out=pout[:, 2 * q * Q : 2 * q * Q + Q], in_=dst[:, 0:Q]
                )
                nc.vector.tensor_scalar(
                    dst[:, Q : 2 * Q],
                    cd[:, Q : 2 * Q],
                    b2sb,
                    0.0,
                    add_op,
                    max_op,
                )
                nc.sync.dma_start(
                    out=pout[:, 2 * q * Q + Q : 2 * (q + 1) * Q],
                    in_=dst[:, Q : 2 * Q],
                )
            else:
                nc.vector.tensor_scalar(dst, cd, b2sb, 0.0, add_op, max_op)
                finish_quantum(q, dst, pc, h2r)

            xc = xc_n
            ab = ab_n

    nc.compile()
    return nc


def _get_program(c4: int):
    if c4 not in _BUILD_CACHE:
        _BUILD_CACHE[c4] = _build_program(c4)
    return _BUILD_CACHE[c4]


def kernel(x, batch, num_graphs, W_in, b_in, W_h, b_h, W_out, b_out):
    from concourse import bass_utils

    x = np.asarray(x, dtype=np.float32)
    batch = np.asarray(batch).astype(np.int64)
    g_total = int(num_graphs)
    W_in = np.asarray(W_in, dtype=np.float32)
    b_in = np.asarray(b_in, dtype=np.float32)
    W_h = np.asarray(W_h, dtype=np.float32)
    b_h = np.asarray(b_h, dtype=np.float32)
    W_out = np.asarray(W_out, dtype=np.float32)
    b_out = np.asarray(b_out, dtype=np.float32)

    if batch.size and np.any(np.diff(batch) < 0):
        order = np.argsort(batch, kind="stable")
        x = x[order]
        batch = batch[order]

    n_nodes, f_in = x.shape
    h_dim = W_in.shape[1]
    assert f_in == F_IN and h_dim == H_DIM

    counts = np.bincount(batch, minlength=g_total).astype(np.int64)
    node_starts = np.concatenate([[0], np.cumsum(counts)])  # [G+1]

    # Per-graph padded counts (multiple of PACK).
    pc_counts = (counts + PACK - 1) // PACK * PACK

    # Balanced graph->core assignment (LPT): the SPMD program runs every core
    # for the max core's column count, so minimizing the max load saves whole
    # quanta versus contiguous 1024-graph shards (~0.6% spread).
    import heapq

    order_g = np.argsort(pc_counts, kind="stable")[::-1]
    heap = [(0, c) for c in range(N_CORES)]
    heapq.heapify(heap)
    core_graphs = [[] for _ in range(N_CORES)]
    loads = [0] * N_CORES
    for g in order_g:
        load, c = heapq.heappop(heap)
        core_graphs[c].append(g)
        load += int(pc_counts[g])
        loads[c] = load
        heapq.heappush(heap, (load, c))
    core_graphs = [np.asarray(gs, dtype=np.int64) for gs in core_graphs]
    c4_per_core = [t // PACK for t in loads]
    align = Q_COLS  # partial DMA batches are supported, so quantum-aligned
    c4 = max(c4_per_core)
    c4 = (c4 + align - 1) // align * align

    # Constant tensors shared by all cores.
    w1blk = np.zeros((128, 128), dtype=np.float32)
    w1blk[0:32, 0:64] = W_in
    w1blk[32:64, 64:128] = W_in
    w1blk[64:96, 0:64] = W_in
    w1blk[96:128, 64:128] = W_in
    w1blk = w1blk.astype(ml_dtypes.bfloat16)
    w2blk = np.zeros((128, 128), dtype=np.float32)
    w2blk[0:64, 0:64] = W_h
    w2blk[64:128, 64:128] = W_h
    w2blk = w2blk.astype(ml_dtypes.bfloat16)
    b1cat = np.tile(b_in, 2).reshape(128, 1).astype(np.float32)
    b2cat = np.tile(b_h, 2).reshape(128, 1).astype(np.float32)

    # Per-core packed inputs.
    in_maps = []
    for c in range(N_CORES):
        gc = core_graphs[c]
        cnt_c = counts[gc]
        pc_c = pc_counts[gc]
        pad_starts = np.concatenate([[0], np.cumsum(pc_c)])  # [len(gc)+1]

        x_padded = np.zeros((c4 * PACK, f_in), dtype=np.float32)
        tot = int(cnt_c.sum())
        if tot > 0:
            # Gather this core's nodes graph-by-graph into padded slots.
            within = np.arange(tot) - np.repeat(
                np.concatenate([[0], np.cumsum(cnt_c)])[:-1], cnt_c
            )
            src = np.repeat(node_starts[gc], cnt_c) + within
            dst = np.repeat(pad_starts[:-1], cnt_c) + within
            x_padded[dst] = x[src]
        xT_dev = (
            x_padded.reshape(c4, PACK, f_in).transpose(1, 2, 0).reshape(128, c4)
        )
        xdt_np = ml_dtypes.float8_e4m3 if X_FP8 else ml_dtypes.bfloat16
        xT_dev = np.ascontiguousarray(xT_dev).astype(xdt_np)
        in_maps.append(
            {
                "xT": xT_dev,
                "w1": w1blk,
                "w2": w2blk,
                "b1": b1cat,
                "b2": b2cat,
            }
        )

    global _LAST_IN_MAPS
    _LAST_IN_MAPS = in_maps

    nc = _get_program(c4)
    res = bass_utils.run_bass_kernel_spmd(
        nc, in_maps, core_ids=list(range(N_CORES))
    )

    # Pad-node contribution, exactly as the device computes it for x=0 rows:
    # h1raw = 0 (fp32 psum) -> D1: bf16(relu(b1)) -> S2 bf16 matmul (fp32 acc)
    # -> D2: bf16(relu(. + b2)).
    bf = ml_dtypes.bfloat16
    h1p = np.maximum(b_in, 0.0).astype(bf).astype(np.float32)
    w2bf = W_h.astype(bf).astype(np.float32)
    vpad = np.maximum(h1p @ w2bf + b_h, 0.0).astype(bf).astype(np.float32)
    vpad_bf = vpad

    out = np.zeros((g_total, W_out.shape[1]), dtype=np.float32)
    for c in range(N_CORES):
        gc = core_graphs[c]
        cnt_c = counts[gc].astype(np.float64)
        pc_c = pc_counts[gc]
        pad_starts = np.concatenate([[0], np.cumsum(pc_c)])
        col_starts = pad_starts // PACK  # [len(gc)+1]

        nq = c4 // Q_COLS
        P = np.asarray(res.results[c]["pout"]).astype(np.float32)
        if DEVICE_R:
            R1 = P[0:64, :] + P[64:128, :]  # [64, c4]
        else:
            Pr = P.reshape(128, nq, 2, Q_COLS)
            # fold partition halves (node pairs) and streams
            R1 = (Pr[0:64] + Pr[64:128]).sum(axis=2).reshape(64, c4)
        cs = np.concatenate(
            [np.zeros((64, 1)), np.cumsum(R1.astype(np.float64), axis=1)], axis=1
        )  # [64, c4+1]
        seg_sum = (cs[:, col_starts[1:]] - cs[:, col_starts[:-1]]).T  # [g, 64]

        n_pad = (pc_c - counts[gc]).astype(np.float64)
        seg_sum = seg_sum - n_pad[:, None] * vpad_bf[None, :].astype(np.float64)
        denom = np.maximum(cnt_c, 1.0)
        mean = seg_sum / denom[:, None]
        mean[cnt_c == 0] = 0.0
        out[gc] = mean.astype(np.float32) @ W_out + b_out

    return out



# revision 59
# speedup vs baseline: 1.0014x; 1.0009x over previous
"""Trainium2 Bass kernel for MockGCN segment-reduce problem.

Pipeline (8-way data-parallel, graphs LPT-balanced across cores so every
core has ~identical column counts):
  h1 = relu(x @ W_in + b_in)         [N, 64]
  h2 = relu(h1 @ W_h + b_h)          [N, 64]
  pooled[g] = mean_{i in g} h2[i]    [G, 64]
  out = pooled @ W_out + b_out       [G, 5]

Device layout: features-on-partitions ("T orientation"), nodes 2-packed
across the 128 partitions (64 feats x 2 node streams) and 4-interleaved
along the free axis so a column of the partial tensor holds 4 consecutive
nodes.  The host pads every segment to a multiple of 4 nodes, packs
x into xT_dev [128, C4] float8_e4m3 (halves input DMA traffic; the
~0.9% extra absmax error is well inside the 2e-2 gate), and the device
emits per-2-node relu'd h2 pairs [128, 2*C4] bf16.  The host folds the
two node streams, does the per-segment cumsum combine, mean division,
pad correction, and the tiny [G,64]@[64,5] matmul.

Stage map (per 512-col quantum = 2048 nodes):
  DMA in xT chunk [128, 1024] fp8 (2 quanta per dma_start; biases ride
      the SWDGE ring in parallel with x0/w1/w2 on HWDGE at startup)
  PE:  warm-up dummy matmuls first (p-state ramps to 2.4GHz before real
       work), then MM-A/MM-B (row-tiled, fp8 x bf16) -> psum ab [128,1024]
  D1:  relu(ab + b1) -> rhs2 [128,1024] bf16 SBUF  (ScalarE, 1038ns)
  PE:  MM x2 (K=128, blkdiag(W2,W2)) -> psum cd [128,1024]
  D2:  relu(cd + b2) -> h2r [128,1024] bf16 SBUF   (VectorE 1192ns;
       ScalarE steals every 14th quantum plus tail quanta to balance)
  DMA out h2r -> pout[:, 2qQ:2(q+1)Q] (2-quantum batches; per-quantum
      flushes for the last 2 quanta shorten the end chain)

The PSUM->SBUF drains (D1/D2) are the hard bottleneck: 32M psum fp32
elements/core can only pass through ScalarE (0.833ns/col + 185ns/instr)
and VectorE (1.042ns/col + 125ns/instr) - Pool/gpsimd and DMA cannot
read PSUM, PSUM's 8 banks cap drain instruction size at 1024 cols, and
DVE 2x modes require 2-byte operands (PSUM is fp32-only on TRN2).
Balanced floor ~1109ns/quantum; this schedule reaches ~1190 steady +
~4.4us startup + ~4us tail = 152.0us (vs 158.3us for the previous
device-side-fold version).
"""

import sys

if "/opt/trn_rl_repo" not in sys.path:
    sys.path.insert(0, "/opt/trn_rl_repo")

from contextlib import ExitStack

import ml_dtypes
import numpy as np

N_CORES = 8
G_TOTAL = 8192
F_IN = 32
H_DIM = 64
Q_COLS = 512  # partial columns per quantum
PACK = 4  # nodes per partial column
DMA_BATCH = 2  # quanta per input DMA transfer
OUT_BATCH = 2  # quanta per output DMA transfer
D2_ACT_EVERY = 14  # ScalarE steals D2 from VectorE every k-th quantum
DEVICE_R = False  # sum the two node streams on device vs on host
DELAY_STEAL = False  # emit the ScalarE-stolen D2 one quantum late
R_ON_POOL = True  # device stream-sum engine: GpSimd (True) or VectorE (False)
X_FP8 = True  # stream x as float8_e4m3 (halves input DMA traffic)
WEIGHTS_ON_SYNC = True  # weights ride HWDGE behind the first x chunk
TAIL_WINDOW = 4  # in the last K quanta, ScalarE takes extra D2 steals
TAIL_EVERY = 2  # ...every k-th quantum of the tail window
PE_WARM_MM = 4  # dummy matmuls to ramp the PE p-state before real work
LAST_R_ON_DVE = True  # final quantum's stream-fold on VectorE (shorter tail)
HALF_STEAL = False  # steal half a D2 every D2_ACT_EVERY//2 quanta instead
TAIL_FLUSH = 2  # flush output per-quantum for the last K quanta
RAMP_SPLIT_D1 = 0  # split the first K quanta's D1 across both drain engines
LAST_D2_SPLIT = False  # split the final D2 + flush per half (shorter end chain)
RAGGED_LAST = True  # final quantum processes only the real columns (no pad)

_BUILD_CACHE: dict = {}
_LAST_IN_MAPS: list | None = None


def _build_program(c4: int, lastq: int = Q_COLS):
    """Build + compile the 8-core SPMD Bass program for C4 partial columns.

    lastq < Q_COLS makes the final quantum ragged: its instructions cover
    only the real columns, trimming the tail-critical last drains/flush.
    """
    import concourse.tile as tile
    from concourse import bacc, mybir

    f32 = mybir.dt.float32
    bf16 = mybir.dt.bfloat16
    xdt = mybir.dt.float8e4 if X_FP8 else bf16
    Relu = mybir.ActivationFunctionType.Relu
    add_op = mybir.AluOpType.add
    max_op = mybir.AluOpType.max

    nq = c4 // Q_COLS
    assert c4 % Q_COLS == 0
    assert 0 < lastq <= Q_COLS
    # A ragged final quantum relies on the per-quantum tail-flush path (the
    # batched flush would interleave garbage columns into the output).
    assert lastq == Q_COLS or (TAIL_FLUSH >= 1 and not DEVICE_R)
    total_cols = (nq - 1) * Q_COLS + lastq

    nc = bacc.Bacc(
        "TRN2",
        target_bir_lowering=False,
        debug=False,
        enable_asserts=False,
        num_devices=N_CORES,
    )

    xT = nc.dram_tensor("xT", [128, c4], xdt, kind="ExternalInput").ap()
    w1 = nc.dram_tensor("w1", [128, 128], bf16, kind="ExternalInput").ap()
    w2 = nc.dram_tensor("w2", [128, 128], bf16, kind="ExternalInput").ap()
    b1 = nc.dram_tensor("b1", [128, 1], f32, kind="ExternalInput").ap()
    b2 = nc.dram_tensor("b2", [128, 1], f32, kind="ExternalInput").ap()
    pw = c4 if DEVICE_R else 2 * c4
    pout = nc.dram_tensor("pout", [128, pw], bf16, kind="ExternalOutput").ap()

    with ExitStack() as ctx:
        tc = ctx.enter_context(tile.TileContext(nc))
        singles = ctx.enter_context(tc.tile_pool(name="singles", bufs=1))
        xpool = ctx.enter_context(tc.tile_pool(name="xc", bufs=3))
        rpool = ctx.enter_context(tc.tile_pool(name="rhs2", bufs=3))
        hpool = ctx.enter_context(tc.tile_pool(name="h2r", bufs=3))
        ppool = ctx.enter_context(tc.tile_pool(name="pc", bufs=3))
        abpool = ctx.enter_context(tc.tile_pool(name="ab", bufs=2, space="PSUM"))
        cdpool = ctx.enter_context(tc.tile_pool(name="cd", bufs=2, space="PSUM"))

        w1sb = singles.tile([128, 128], bf16)
        w2sb = singles.tile([128, 128], bf16)
        b1sb = singles.tile([128, 1], f32)
        b2sb = singles.tile([128, 1], f32)
        Q = Q_COLS
        xc0 = None
        if WEIGHTS_ON_SYNC:
            # Biases ride the (otherwise idle) SWDGE ring in parallel; the
            # first x chunk leads the HWDGE FIFO with w1/w2 right behind it.
            nc.gpsimd.dma_start(out=w1sb, in_=w1)
            nc.gpsimd.dma_start(out=b1sb, in_=b1)
            nc.gpsimd.dma_start(out=b2sb, in_=b2)
            # Chunk 0 is split so quantum 0's columns land in one minimal
            # transfer; the rest of the chunk follows behind the weights.
            take0 = min(DMA_BATCH, nq)
            xc0 = xpool.tile([128, DMA_BATCH * Q], xdt, name="xc")
            end00 = min(Q, total_cols)
            nc.sync.dma_start(out=xc0[:, 0:end00], in_=xT[:, 0:end00])
            if take0 > 1:
                end0 = min(take0 * Q, total_cols)
                nc.sync.dma_start(
                    out=xc0[:, Q:end0], in_=xT[:, Q:end0]
                )
            nc.sync.dma_start(out=w2sb, in_=w2)
        else:
            nc.gpsimd.dma_start(out=w1sb, in_=w1)
            nc.gpsimd.dma_start(out=w2sb, in_=w2)
            nc.gpsimd.dma_start(out=b1sb, in_=b1)
            nc.gpsimd.dma_start(out=b2sb, in_=b2)

        # Pre-warm the ScalarE activation table (~2.7us PSEUDO_LOAD_ACT_FUNC_SET
        # attaches to the first ACTIVATE) so it overlaps the first x-chunk DMA.
        warm = singles.tile([128, 1], f32)
        nc.vector.memset(warm, 0.0)
        nc.scalar.activation(warm, warm, Relu)

        # Pre-warm the PE p-state: dummy matmuls on a zeroed scratch tile keep
        # the PE continuously busy while the first x chunk is in flight, so
        # real matmuls start at the full 2.4GHz clock instead of 1.2GHz.
        ab0 = None
        if PE_WARM_MM:
            scratch = singles.tile([128, 512], bf16)
            nc.vector.memset(scratch, 0.0)
            ab0 = abpool.tile([128, 2 * Q_COLS], f32, name="ab")
            for _ in range(PE_WARM_MM):
                nc.tensor.matmul(
                    out=ab0[:, 0:Q_COLS],
                    lhsT=scratch[:, 0:128],
                    rhs=scratch,
                    start=True,
                    stop=True,
                )

        # Software-pipelined PE stream: stage-1 of quantum q+1 is emitted
        # BEFORE stage-2 of quantum q, so the in-order PE queue never blocks
        # the next D1's input behind a stage-2 that waits on the current D1.
        # This breaks the D1->stage2->stage1->D1 dependency cycle (~1209ns)
        # that otherwise paces the steady state above the engine-busy floor.
        xc = None
        h2r = None
        pc = None
        pending = None  # delayed ScalarE-stolen D2: (cd, dst, pc, h2r, q)

        def qcols(q):
            return lastq if q == nq - 1 else Q

        def pair_view(t, qc):
            # Two stream windows of qc columns at offsets 0 and Q.
            if qc == Q:
                return t
            return t.rearrange("p (s q) -> p s q", s=2)[:, :, 0:qc]

        def fetch_chunk(q):
            take = min(DMA_BATCH, nq - q)
            end = min((q + take) * Q, total_cols)
            t = xpool.tile([128, DMA_BATCH * Q], xdt, name="xc")
            nc.sync.dma_start(
                out=t[:, 0 : end - q * Q], in_=xT[:, q * Q : end]
            )
            return t

        def stage1(q, xc_t, ab_t):
            jx = (q % DMA_BATCH) * Q
            qc = qcols(q)
            nc.tensor.matmul(
                out=ab_t[:, 0:qc],
                lhsT=w1sb[0:64, :],
                rhs=xc_t[0:64, jx : jx + qc],
                start=True,
                stop=True,
            )
            nc.tensor.matmul(
                out=ab_t[:, Q : Q + qc],
                lhsT=w1sb[64:128, :],
                rhs=xc_t[64:128, jx : jx + qc],
                start=True,
                stop=True,
            )

        # Prologue: quantum 0's stage-1 runs right after the PE warm-up.
        xc = xc0 if xc0 is not None else fetch_chunk(0)
        ab = ab0 if ab0 is not None else abpool.tile([128, 2 * Q], f32, name="ab")
        stage1(0, xc, ab)

        for q in range(nq):
            if q % OUT_BATCH == 0:
                h2r = hpool.tile([128, OUT_BATCH * 2 * Q], bf16)
            jh = (q % OUT_BATCH) * 2 * Q

            # Prefetch + stage-1 for quantum q+1 (ahead of this quantum's
            # stage-2 in the PE queue).
            if q + 1 < nq:
                if (q + 1) % DMA_BATCH == 0:
                    xc_n = fetch_chunk(q + 1)
                else:
                    xc_n = xc
                ab_n = abpool.tile([128, 2 * Q], f32, name="ab")
                stage1(q + 1, xc_n, ab_n)
            else:
                xc_n = None
                ab_n = None

            qc = qcols(q)
            rhs2 = rpool.tile([128, 2 * Q], bf16)
            if qc < Q:
                nc.scalar.activation(
                    pair_view(rhs2, qc), pair_view(ab, qc), Relu, bias=b1sb
                )
            elif q < RAMP_SPLIT_D1:
                # Pipeline fill: VectorE is idle until its first D2 arrives,
                # so splitting the first D1s across both engines halves the
                # serial latency of the pipe-fill chain.
                nc.scalar.activation(
                    rhs2[:, 0:Q], ab[:, 0:Q], Relu, bias=b1sb
                )
                nc.vector.tensor_scalar(
                    rhs2[:, Q : 2 * Q],
                    ab[:, Q : 2 * Q],
                    b1sb,
                    0.0,
                    add_op,
                    max_op,
                )
            else:
                nc.scalar.activation(rhs2, ab, Relu, bias=b1sb)

            def finish_quantum(qq, dst_t, pc_t, h2r_t, fold_on_dve=False):
                if DEVICE_R:
                    r_eng = (
                        nc.vector
                        if (
                            fold_on_dve
                            or not R_ON_POOL
                            or (LAST_R_ON_DVE and qq == nq - 1)
                        )
                        else nc.gpsimd
                    )
                    r_eng.tensor_add(
                        pc_t[:, (qq % OUT_BATCH) * Q : (qq % OUT_BATCH + 1) * Q],
                        dst_t[:, 0:Q],
                        dst_t[:, Q : 2 * Q],
                    )
                tfs = ((nq - TAIL_FLUSH) // OUT_BATCH) * OUT_BATCH  # batch-aligned
                if TAIL_FLUSH > 0 and qq >= tfs:
                    # Tail: flush each quantum's output as soon as it lands,
                    # so the final DMA chain is short.
                    j = qq % OUT_BATCH
                    if DEVICE_R:
                        nc.sync.dma_start(
                            out=pout[:, qq * Q : (qq + 1) * Q],
                            in_=pc_t[:, j * Q : (j + 1) * Q],
                        )
                    else:
                        qcq = qcols(qq)
                        nc.sync.dma_start(
                            out=pout[:, 2 * qq * Q : 2 * qq * Q + 2 * qcq],
                            in_=pair_view(dst_t, qcq),
                        )
                elif qq % OUT_BATCH == OUT_BATCH - 1 or qq == nq - 1:
                    nb = qq % OUT_BATCH + 1  # quanta in this (possibly short) batch
                    q0 = qq - (nb - 1)
                    if DEVICE_R:
                        nc.sync.dma_start(
                            out=pout[:, q0 * Q : (qq + 1) * Q],
                            in_=pc_t[:, 0 : nb * Q],
                        )
                    else:
                        nc.sync.dma_start(
                            out=pout[:, 2 * q0 * Q : 2 * (qq + 1) * Q],
                            in_=h2r_t[:, 0 : nb * 2 * Q],
                        )

            # Flush a D2 stolen by ScalarE one quantum ago (its deps are long
            # ready, so ScalarE never stalls on this quantum's stage-2).
            if pending is not None:
                cd_p, dst_p, pc_p, h2r_p, q_p = pending
                nc.scalar.activation(dst_p, cd_p, Relu, bias=b2sb)
                finish_quantum(q_p, dst_p, pc_p, h2r_p)
                pending = None

            cd = cdpool.tile([128, 2 * Q], f32)
            nc.tensor.matmul(
                out=cd[:, 0:qc], lhsT=w2sb, rhs=rhs2[:, 0:qc], start=True, stop=True
            )
            nc.tensor.matmul(
                out=cd[:, Q : Q + qc],
                lhsT=w2sb,
                rhs=rhs2[:, Q : Q + qc],
                start=True,
                stop=True,
            )

            if DEVICE_R and q % OUT_BATCH == 0:
                pc = ppool.tile([128, OUT_BATCH * Q], bf16)
            dst = h2r[:, jh : jh + 2 * Q]
            steady_steal = (
                q % D2_ACT_EVERY == D2_ACT_EVERY - 1
                and q % OUT_BATCH == OUT_BATCH - 1
                and q < nq - 1
            )
            half_steal = (
                HALF_STEAL
                and q % (D2_ACT_EVERY // 2) == D2_ACT_EVERY // 2 - 1
                and q < nq - 1
            )
            tail_steal = (
                q >= nq - TAIL_WINDOW and q % TAIL_EVERY == TAIL_EVERY - 1
            )
            if HALF_STEAL and not tail_steal:
                if half_steal:
                    nc.scalar.activation(
                        dst[:, 0:Q], cd[:, 0:Q], Relu, bias=b2sb
                    )
                    nc.vector.tensor_scalar(
                        dst[:, Q : 2 * Q],
                        cd[:, Q : 2 * Q],
                        b2sb,
                        0.0,
                        add_op,
                        max_op,
                    )
                else:
                    nc.vector.tensor_scalar(dst, cd, b2sb, 0.0, add_op, max_op)
                finish_quantum(q, dst, pc, h2r)
            elif steady_steal or tail_steal:
                if DELAY_STEAL:
                    pending = (cd, dst, pc, h2r, q)
                else:
                    nc.scalar.activation(
                        pair_view(dst, qc), pair_view(cd, qc), Relu, bias=b2sb
                    )
                    finish_quantum(q, dst, pc, h2r)
            elif LAST_D2_SPLIT and q == nq - 1 and not DEVICE_R:
                # Final quantum: drain + flush in halves so the first half's
                # output DMA overlaps the second half's drain.
                nc.vector.tensor_scalar(
                    dst[:, 0:Q], cd[:, 0:Q], b2sb, 0.0, add_op, max_op
                )
                nc.sync.dma_start(
                    out=pout[:, 2 * q * Q : 2 * q * Q + Q], in_=dst[:, 0:Q]
                )
                nc.vector.tensor_scalar(
                    dst[:, Q : 2 * Q],
                    cd[:, Q : 2 * Q],
                    b2sb,
                    0.0,
                    add_op,
                    max_op,
                )
                nc.sync.dma_start(
                    out=pout[:, 2 * q * Q + Q : 2 * (q + 1) * Q],
                    in_=dst[:, Q : 2 * Q],
                )
            else:
                nc.vector.tensor_scalar(
                    pair_view(dst, qc), pair_view(cd, qc), b2sb, 0.0, add_op, max_op
                )
                finish_quantum(q, dst, pc, h2r)

            xc = xc_n
            ab = ab_n

    nc.compile()
    return nc


def _get_program(c4: int, lastq: int = Q_COLS):
    key = (c4, lastq)
    if key not in _BUILD_CACHE:
        _BUILD_CACHE[key] = _build_program(c4, lastq)
    return _BUILD_CACHE[key]


def kernel(x, batch, num_graphs, W_in, b_in, W_h, b_h, W_out, b_out):
    from concourse import bass_utils

    x = np.asarray(x, dtype=np.float32)
    batch = np.asarray(batch).astype(np.int64)
    g_total = int(num_graphs)
    W_in = np.asarray(W_in, dtype=np.float32)
    b_in = np.asarray(b_in, dtype=np.float32)
    W_h = np.asarray(W_h, dtype=np.float32)
    b_h = np.asarray(b_h, dtype=np.float32)
    W_out = np.asarray(W_out, dtype=np.float32)
    b_out = np.asarray(b_out, dtype=np.float32)

    if batch.size and np.any(np.diff(batch) < 0):
        order = np.argsort(batch, kind="stable")
        x = x[order]
        batch = batch[order]

    n_nodes, f_in = x.shape
    h_dim = W_in.shape[1]
    assert f_in == F_IN and h_dim == H_DIM

    counts = np.bincount(batch, minlength=g_total).astype(np.int64)
    node_starts = np.concatenate([[0], np.cumsum(counts)])  # [G+1]

    # Per-graph padded counts (multiple of PACK).
    pc_counts = (counts + PACK - 1) // PACK * PACK

    # Balanced graph->core assignment (LPT): the SPMD program runs every core
    # for the max core's column count, so minimizing the max load saves whole
    # quanta versus contiguous 1024-graph shards (~0.6% spread).
    import heapq

    order_g = np.argsort(pc_counts, kind="stable")[::-1]
    heap = [(0, c) for c in range(N_CORES)]
    heapq.heapify(heap)
    core_graphs = [[] for _ in range(N_CORES)]
    loads = [0] * N_CORES
    for g in order_g:
        load, c = heapq.heappop(heap)
        core_graphs[c].append(g)
        load += int(pc_counts[g])
        loads[c] = load
        heapq.heappush(heap, (load, c))
    core_graphs = [np.asarray(gs, dtype=np.int64) for gs in core_graphs]
    c4_per_core = [t // PACK for t in loads]
    c_exact = max(c4_per_core)  # exact max columns across cores
    nq_h = (c_exact + Q_COLS - 1) // Q_COLS
    c4 = nq_h * Q_COLS
    lastq = (
        c_exact - (nq_h - 1) * Q_COLS
        if (RAGGED_LAST and c_exact > (nq_h - 1) * Q_COLS)
        else Q_COLS
    )

    # Constant tensors shared by all cores.
    w1blk = np.zeros((128, 128), dtype=np.float32)
    w1blk[0:32, 0:64] = W_in
    w1blk[32:64, 64:128] = W_in
    w1blk[64:96, 0:64] = W_in
    w1blk[96:128, 64:128] = W_in
    w1blk = w1blk.astype(ml_dtypes.bfloat16)
    w2blk = np.zeros((128, 128), dtype=np.float32)
    w2blk[0:64, 0:64] = W_h
    w2blk[64:128, 64:128] = W_h
    w2blk = w2blk.astype(ml_dtypes.bfloat16)
    b1cat = np.tile(b_in, 2).reshape(128, 1).astype(np.float32)
    b2cat = np.tile(b_h, 2).reshape(128, 1).astype(np.float32)

    # Per-core packed inputs.
    in_maps = []
    for c in range(N_CORES):
        gc = core_graphs[c]
        cnt_c = counts[gc]
        pc_c = pc_counts[gc]
        pad_starts = np.concatenate([[0], np.cumsum(pc_c)])  # [len(gc)+1]

        x_padded = np.zeros((c4 * PACK, f_in), dtype=np.float32)
        tot = int(cnt_c.sum())
        if tot > 0:
            # Gather this core's nodes graph-by-graph into padded slots.
            within = np.arange(tot) - np.repeat(
                np.concatenate([[0], np.cumsum(cnt_c)])[:-1], cnt_c
            )
            src = np.repeat(node_starts[gc], cnt_c) + within
            dst = np.repeat(pad_starts[:-1], cnt_c) + within
            x_padded[dst] = x[src]
        xT_dev = (
            x_padded.reshape(c4, PACK, f_in).transpose(1, 2, 0).reshape(128, c4)
        )
        xdt_np = ml_dtypes.float8_e4m3 if X_FP8 else ml_dtypes.bfloat16
        xT_dev = np.ascontiguousarray(xT_dev).astype(xdt_np)
        in_maps.append(
            {
                "xT": xT_dev,
                "w1": w1blk,
                "w2": w2blk,
                "b1": b1cat,
                "b2": b2cat,
            }
        )

    global _LAST_IN_MAPS
    _LAST_IN_MAPS = in_maps

    nc = _get_program(c4, lastq)
    res = bass_utils.run_bass_kernel_spmd(
        nc, in_maps, core_ids=list(range(N_CORES))
    )

    # Pad-node contribution, exactly as the device computes it for x=0 rows:
    # h1raw = 0 (fp32 psum) -> D1: bf16(relu(b1)) -> S2 bf16 matmul (fp32 acc)
    # -> D2: bf16(relu(. + b2)).
    bf = ml_dtypes.bfloat16
    h1p = np.maximum(b_in, 0.0).astype(bf).astype(np.float32)
    w2bf = W_h.astype(bf).astype(np.float32)
    vpad = np.maximum(h1p @ w2bf + b_h, 0.0).astype(bf).astype(np.float32)
    vpad_bf = vpad

    out = np.zeros((g_total, W_out.shape[1]), dtype=np.float32)
    for c in range(N_CORES):
        gc = core_graphs[c]
        cnt_c = counts[gc].astype(np.float64)
        pc_c = pc_counts[gc]
        pad_starts = np.concatenate([[0], np.cumsum(pc_c)])
        col_starts = pad_starts // PACK  # [len(gc)+1]

        nq = c4 // Q_COLS
        P = np.asarray(res.results[c]["pout"]).astype(np.float32)
        if DEVICE_R:
            R1 = P[0:64, :] + P[64:128, :]  # [64, c4]
        else:
            # fold partition halves (node pairs) and streams; the final
            # quantum may be ragged (lastq < Q_COLS real columns).
            nf = nq - 1
            Pf = P[:, : nf * 2 * Q_COLS].reshape(128, nf, 2, Q_COLS)
            R1f = (Pf[0:64] + Pf[64:128]).sum(axis=2).reshape(
                64, nf * Q_COLS
            )
            Pl = P[
                :, nf * 2 * Q_COLS : nf * 2 * Q_COLS + 2 * lastq
            ].reshape(128, 2, lastq)
            R1l = (Pl[0:64] + Pl[64:128]).sum(axis=1)
            R1 = np.concatenate([R1f, R1l], axis=1)  # [64, c_exact]
        cs = np.concatenate(
            [np.zeros((64, 1)), np.cumsum(R1.astype(np.float64), axis=1)], axis=1
        )  # [64, c4+1]
        seg_sum = (cs[:, col_starts[1:]] - cs[:, col_starts[:-1]]).T  # [g, 64]

        n_pad = (pc_c - counts[gc]).astype(np.float64)
        seg_sum = seg_sum - n_pad[:, None] * vpad_bf[None, :].astype(np.float64)
        denom = np.maximum(cnt_c, 1.0)
        mean = seg_sum / denom[:, None]
        mean[cnt_c == 0] = 0.0
        out[gc] = mean.astype(np.float32) @ W_out + b_out

    return out



# revision 62
# speedup vs baseline: 1.0019x; 1.0005x over previous
"""Trainium2 Bass kernel for MockGCN segment-reduce problem.

Pipeline (8-way data-parallel, graphs LPT-balanced across cores so every
core has ~identical column counts):
  h1 = relu(x @ W_in + b_in)         [N, 64]
  h2 = relu(h1 @ W_h + b_h)          [N, 64]
  pooled[g] = mean_{i in g} h2[i]    [G, 64]
  out = pooled @ W_out + b_out       [G, 5]

Device layout: features-on-partitions ("T orientation"), nodes 2-packed
across the 128 partitions (64 feats x 2 node streams) and 4-interleaved
along the free axis so a column of the partial tensor holds 4 consecutive
nodes.  The host pads every segment to a multiple of 4 nodes, packs
x into xT_dev [128, C4] float8_e4m3 (halves input DMA traffic; the
~0.9% extra absmax error is well inside the 2e-2 gate), and the device
emits per-2-node relu'd h2 pairs [128, 2*C4] bf16.  The host folds the
two node streams, does the per-segment cumsum combine, mean division,
pad correction, and the tiny [G,64]@[64,5] matmul.

Stage map (per 512-col quantum = 2048 nodes):
  DMA in xT chunk [128, 1024] fp8 (2 quanta per dma_start; biases ride
      the SWDGE ring in parallel with x0/w1/w2 on HWDGE at startup)
  PE:  warm-up dummy matmuls first (p-state ramps to 2.4GHz before real
       work), then MM-A/MM-B (row-tiled, fp8 x bf16) -> psum ab [128,1024]
  D1:  relu(ab + b1) -> rhs2 [128,1024] bf16 SBUF  (ScalarE, 1038ns)
  PE:  MM x2 (K=128, blkdiag(W2,W2)) -> psum cd [128,1024]
  D2:  relu(cd + b2) -> h2r [128,1024] bf16 SBUF   (VectorE 1192ns;
       ScalarE steals every 14th quantum plus tail quanta to balance)
  DMA out h2r -> pout[:, 2qQ:2(q+1)Q] (2-quantum batches; per-quantum
      flushes for the last 2 quanta shorten the end chain)

The PSUM->SBUF drains (D1/D2) are the hard bottleneck: 32M psum fp32
elements/core can only pass through ScalarE (0.833ns/col + 185ns/instr)
and VectorE (1.042ns/col + 125ns/instr) - Pool/gpsimd and DMA cannot
read PSUM, PSUM's 8 banks cap drain instruction size at 1024 cols, and
DVE 2x modes require 2-byte operands (PSUM is fp32-only on TRN2).
Balanced floor ~1109ns/quantum; this schedule reaches ~1190 steady +
~4.4us startup + ~3.7us tail = 151.9us (vs 158.3us for the previous
device-side-fold version).  The final quantum is ragged (only the real
columns, e.g. 417 of 512), trimming the tail-critical last drains.
"""

import sys

if "/opt/trn_rl_repo" not in sys.path:
    sys.path.insert(0, "/opt/trn_rl_repo")

from contextlib import ExitStack

import ml_dtypes
import numpy as np

N_CORES = 8
G_TOTAL = 8192
F_IN = 32
H_DIM = 64
Q_COLS = 512  # partial columns per quantum
PACK = 4  # nodes per partial column
DMA_BATCH = 2  # quanta per input DMA transfer
OUT_BATCH = 2  # quanta per output DMA transfer
D2_ACT_EVERY = 14  # ScalarE steals D2 from VectorE every k-th quantum
STEAL_OFFSET = 11  # phase of the steal within the 14-quantum cycle (odd)
DEVICE_R = False  # sum the two node streams on device vs on host
DELAY_STEAL = False  # emit the ScalarE-stolen D2 one quantum late
R_ON_POOL = True  # device stream-sum engine: GpSimd (True) or VectorE (False)
X_FP8 = True  # stream x as float8_e4m3 (halves input DMA traffic)
WEIGHTS_ON_SYNC = True  # weights ride HWDGE behind the first x chunk
TAIL_WINDOW = 4  # in the last K quanta, ScalarE takes extra D2 steals
TAIL_EVERY = 2  # ...every k-th quantum of the tail window
PE_WARM_MM = 4  # dummy matmuls to ramp the PE p-state before real work
LAST_R_ON_DVE = True  # final quantum's stream-fold on VectorE (shorter tail)
HALF_STEAL = False  # steal half a D2 every D2_ACT_EVERY//2 quanta instead
TAIL_FLUSH = 2  # flush output per-quantum for the last K quanta
RAMP_SPLIT_D1 = 0  # split the first K quanta's D1 across both drain engines
LAST_D2_SPLIT = False  # split the final D2 + flush per half (shorter end chain)
RAGGED_LAST = True  # final quantum processes only the real columns (no pad)

_BUILD_CACHE: dict = {}
_LAST_IN_MAPS: list | None = None


def _build_program(c4: int, lastq: int = Q_COLS):
    """Build + compile the 8-core SPMD Bass program for C4 partial columns.

    lastq < Q_COLS makes the final quantum ragged: its instructions cover
    only the real columns, trimming the tail-critical last drains/flush.
    """
    import concourse.tile as tile
    from concourse import bacc, mybir

    f32 = mybir.dt.float32
    bf16 = mybir.dt.bfloat16
    xdt = mybir.dt.float8e4 if X_FP8 else bf16
    Relu = mybir.ActivationFunctionType.Relu
    add_op = mybir.AluOpType.add
    max_op = mybir.AluOpType.max

    nq = c4 // Q_COLS
    assert c4 % Q_COLS == 0
    assert 0 < lastq <= Q_COLS
    # A ragged final quantum relies on the per-quantum tail-flush path (the
    # batched flush would interleave garbage columns into the output).
    assert lastq == Q_COLS or (TAIL_FLUSH >= 1 and not DEVICE_R)
    total_cols = (nq - 1) * Q_COLS + lastq

    nc = bacc.Bacc(
        "TRN2",
        target_bir_lowering=False,
        debug=False,
        enable_asserts=False,
        num_devices=N_CORES,
    )

    xT = nc.dram_tensor("xT", [128, c4], xdt, kind="ExternalInput").ap()
    w1 = nc.dram_tensor("w1", [128, 128], bf16, kind="ExternalInput").ap()
    w2 = nc.dram_tensor("w2", [128, 128], bf16, kind="ExternalInput").ap()
    b1 = nc.dram_tensor("b1", [128, 1], f32, kind="ExternalInput").ap()
    b2 = nc.dram_tensor("b2", [128, 1], f32, kind="ExternalInput").ap()
    pw = c4 if DEVICE_R else 2 * c4
    pout = nc.dram_tensor("pout", [128, pw], bf16, kind="ExternalOutput").ap()

    with ExitStack() as ctx:
        tc = ctx.enter_context(tile.TileContext(nc))
        singles = ctx.enter_context(tc.tile_pool(name="singles", bufs=1))
        xpool = ctx.enter_context(tc.tile_pool(name="xc", bufs=3))
        rpool = ctx.enter_context(tc.tile_pool(name="rhs2", bufs=3))
        hpool = ctx.enter_context(tc.tile_pool(name="h2r", bufs=3))
        ppool = ctx.enter_context(tc.tile_pool(name="pc", bufs=3))
        abpool = ctx.enter_context(tc.tile_pool(name="ab", bufs=2, space="PSUM"))
        cdpool = ctx.enter_context(tc.tile_pool(name="cd", bufs=2, space="PSUM"))

        w1sb = singles.tile([128, 128], bf16)
        w2sb = singles.tile([128, 128], bf16)
        b1sb = singles.tile([128, 1], f32)
        b2sb = singles.tile([128, 1], f32)
        Q = Q_COLS
        xc0 = None
        if WEIGHTS_ON_SYNC:
            # Biases ride the (otherwise idle) SWDGE ring in parallel; the
            # first x chunk leads the HWDGE FIFO with w1/w2 right behind it.
            nc.gpsimd.dma_start(out=w1sb, in_=w1)
            nc.gpsimd.dma_start(out=b1sb, in_=b1)
            nc.gpsimd.dma_start(out=b2sb, in_=b2)
            # Chunk 0 is split so quantum 0's columns land in one minimal
            # transfer; the rest of the chunk follows behind the weights.
            take0 = min(DMA_BATCH, nq)
            xc0 = xpool.tile([128, DMA_BATCH * Q], xdt, name="xc")
            end00 = min(Q, total_cols)
            nc.sync.dma_start(out=xc0[:, 0:end00], in_=xT[:, 0:end00])
            if take0 > 1:
                end0 = min(take0 * Q, total_cols)
                nc.sync.dma_start(
                    out=xc0[:, Q:end0], in_=xT[:, Q:end0]
                )
            nc.sync.dma_start(out=w2sb, in_=w2)
        else:
            nc.gpsimd.dma_start(out=w1sb, in_=w1)
            nc.gpsimd.dma_start(out=w2sb, in_=w2)
            nc.gpsimd.dma_start(out=b1sb, in_=b1)
            nc.gpsimd.dma_start(out=b2sb, in_=b2)

        # Pre-warm the ScalarE activation table (~2.7us PSEUDO_LOAD_ACT_FUNC_SET
        # attaches to the first ACTIVATE) so it overlaps the first x-chunk DMA.
        warm = singles.tile([128, 1], f32)
        nc.vector.memset(warm, 0.0)
        nc.scalar.activation(warm, warm, Relu)

        # Pre-warm the PE p-state: dummy matmuls on a zeroed scratch tile keep
        # the PE continuously busy while the first x chunk is in flight, so
        # real matmuls start at the full 2.4GHz clock instead of 1.2GHz.
        ab0 = None
        if PE_WARM_MM:
            scratch = singles.tile([128, 512], bf16)
            nc.vector.memset(scratch, 0.0)
            ab0 = abpool.tile([128, 2 * Q_COLS], f32, name="ab")
            for _ in range(PE_WARM_MM):
                nc.tensor.matmul(
                    out=ab0[:, 0:Q_COLS],
                    lhsT=scratch[:, 0:128],
                    rhs=scratch,
                    start=True,
                    stop=True,
                )

        # Software-pipelined PE stream: stage-1 of quantum q+1 is emitted
        # BEFORE stage-2 of quantum q, so the in-order PE queue never blocks
        # the next D1's input behind a stage-2 that waits on the current D1.
        # This breaks the D1->stage2->stage1->D1 dependency cycle (~1209ns)
        # that otherwise paces the steady state above the engine-busy floor.
        xc = None
        h2r = None
        pc = None
        pending = None  # delayed ScalarE-stolen D2: (cd, dst, pc, h2r, q)

        def qcols(q):
            return lastq if q == nq - 1 else Q

        def pair_view(t, qc):
            # Two stream windows of qc columns at offsets 0 and Q.
            if qc == Q:
                return t
            return t.rearrange("p (s q) -> p s q", s=2)[:, :, 0:qc]

        def fetch_chunk(q):
            take = min(DMA_BATCH, nq - q)
            end = min((q + take) * Q, total_cols)
            t = xpool.tile([128, DMA_BATCH * Q], xdt, name="xc")
            nc.sync.dma_start(
                out=t[:, 0 : end - q * Q], in_=xT[:, q * Q : end]
            )
            return t

        def stage1(q, xc_t, ab_t):
            jx = (q % DMA_BATCH) * Q
            qc = qcols(q)
            nc.tensor.matmul(
                out=ab_t[:, 0:qc],
                lhsT=w1sb[0:64, :],
                rhs=xc_t[0:64, jx : jx + qc],
                start=True,
                stop=True,
            )
            nc.tensor.matmul(
                out=ab_t[:, Q : Q + qc],
                lhsT=w1sb[64:128, :],
                rhs=xc_t[64:128, jx : jx + qc],
                start=True,
                stop=True,
            )

        # Prologue: quantum 0's stage-1 runs right after the PE warm-up.
        xc = xc0 if xc0 is not None else fetch_chunk(0)
        ab = ab0 if ab0 is not None else abpool.tile([128, 2 * Q], f32, name="ab")
        stage1(0, xc, ab)

        for q in range(nq):
            if q % OUT_BATCH == 0:
                h2r = hpool.tile([128, OUT_BATCH * 2 * Q], bf16)
            jh = (q % OUT_BATCH) * 2 * Q

            # Prefetch + stage-1 for quantum q+1 (ahead of this quantum's
            # stage-2 in the PE queue).
            if q + 1 < nq:
                if (q + 1) % DMA_BATCH == 0:
                    xc_n = fetch_chunk(q + 1)
                else:
                    xc_n = xc
                ab_n = abpool.tile([128, 2 * Q], f32, name="ab")
                stage1(q + 1, xc_n, ab_n)
            else:
                xc_n = None
                ab_n = None

            qc = qcols(q)
            rhs2 = rpool.tile([128, 2 * Q], bf16)
            if qc < Q:
                nc.scalar.activation(
                    pair_view(rhs2, qc), pair_view(ab, qc), Relu, bias=b1sb
                )
            elif q < RAMP_SPLIT_D1:
                # Pipeline fill: VectorE is idle until its first D2 arrives,
                # so splitting the first D1s across both engines halves the
                # serial latency of the pipe-fill chain.
                nc.scalar.activation(
                    rhs2[:, 0:Q], ab[:, 0:Q], Relu, bias=b1sb
                )
                nc.vector.tensor_scalar(
                    rhs2[:, Q : 2 * Q],
                    ab[:, Q : 2 * Q],
                    b1sb,
                    0.0,
                    add_op,
                    max_op,
                )
            else:
                nc.scalar.activation(rhs2, ab, Relu, bias=b1sb)

            def finish_quantum(qq, dst_t, pc_t, h2r_t, fold_on_dve=False):
                if DEVICE_R:
                    r_eng = (
                        nc.vector
                        if (
                            fold_on_dve
                            or not R_ON_POOL
                            or (LAST_R_ON_DVE and qq == nq - 1)
                        )
                        else nc.gpsimd
                    )
                    r_eng.tensor_add(
                        pc_t[:, (qq % OUT_BATCH) * Q : (qq % OUT_BATCH + 1) * Q],
                        dst_t[:, 0:Q],
                        dst_t[:, Q : 2 * Q],
                    )
                tfs = ((nq - TAIL_FLUSH) // OUT_BATCH) * OUT_BATCH  # batch-aligned
                if TAIL_FLUSH > 0 and qq >= tfs:
                    # Tail: flush each quantum's output as soon as it lands,
                    # so the final DMA chain is short.
                    j = qq % OUT_BATCH
                    if DEVICE_R:
                        nc.sync.dma_start(
                            out=pout[:, qq * Q : (qq + 1) * Q],
                            in_=pc_t[:, j * Q : (j + 1) * Q],
                        )
                    else:
                        qcq = qcols(qq)
                        nc.sync.dma_start(
                            out=pout[:, 2 * qq * Q : 2 * qq * Q + 2 * qcq],
                            in_=pair_view(dst_t, qcq),
                        )
                elif qq % OUT_BATCH == OUT_BATCH - 1 or qq == nq - 1:
                    nb = qq % OUT_BATCH + 1  # quanta in this (possibly short) batch
                    q0 = qq - (nb - 1)
                    if DEVICE_R:
                        nc.sync.dma_start(
                            out=pout[:, q0 * Q : (qq + 1) * Q],
                            in_=pc_t[:, 0 : nb * Q],
                        )
                    else:
                        nc.sync.dma_start(
                            out=pout[:, 2 * q0 * Q : 2 * (qq + 1) * Q],
                            in_=h2r_t[:, 0 : nb * 2 * Q],
                        )

            # Flush a D2 stolen by ScalarE one quantum ago (its deps are long
            # ready, so ScalarE never stalls on this quantum's stage-2).
            if pending is not None:
                cd_p, dst_p, pc_p, h2r_p, q_p = pending
                nc.scalar.activation(dst_p, cd_p, Relu, bias=b2sb)
                finish_quantum(q_p, dst_p, pc_p, h2r_p)
                pending = None

            cd = cdpool.tile([128, 2 * Q], f32)
            nc.tensor.matmul(
                out=cd[:, 0:qc], lhsT=w2sb, rhs=rhs2[:, 0:qc], start=True, stop=True
            )
            nc.tensor.matmul(
                out=cd[:, Q : Q + qc],
                lhsT=w2sb,
                rhs=rhs2[:, Q : Q + qc],
                start=True,
                stop=True,
            )

            if DEVICE_R and q % OUT_BATCH == 0:
                pc = ppool.tile([128, OUT_BATCH * Q], bf16)
            dst = h2r[:, jh : jh + 2 * Q]
            steady_steal = (
                q % D2_ACT_EVERY == STEAL_OFFSET
                and q % OUT_BATCH == OUT_BATCH - 1
                and q < nq - 1
            )
            half_steal = (
                HALF_STEAL
                and q % (D2_ACT_EVERY // 2) == D2_ACT_EVERY // 2 - 1
                and q < nq - 1
            )
            tail_steal = (
                q >= nq - TAIL_WINDOW and q % TAIL_EVERY == TAIL_EVERY - 1
            )
            if HALF_STEAL and not tail_steal:
                if half_steal:
                    nc.scalar.activation(
                        dst[:, 0:Q], cd[:, 0:Q], Relu, bias=b2sb
                    )
                    nc.vector.tensor_scalar(
                        dst[:, Q : 2 * Q],
                        cd[:, Q : 2 * Q],
                        b2sb,
                        0.0,
                        add_op,
                        max_op,
                    )
                else:
                    nc.vector.tensor_scalar(dst, cd, b2sb, 0.0, add_op, max_op)
                finish_quantum(q, dst, pc, h2r)
            elif steady_steal or tail_steal:
                if DELAY_STEAL:
                    pending = (cd, dst, pc, h2r, q)
                else:
                    nc.scalar.activation(
                        pair_view(dst, qc), pair_view(cd, qc), Relu, bias=b2sb
                    )
                    finish_quantum(q, dst, pc, h2r)
            elif LAST_D2_SPLIT and q == nq - 1 and not DEVICE_R:
                # Final quantum: drain + flush in halves so the first half's
                # output DMA overlaps the second half's drain.
                nc.vector.tensor_scalar(
                    dst[:, 0:Q], cd[:, 0:Q], b2sb, 0.0, add_op, max_op
                )
                nc.sync.dma_start(
                    out=pout[:, 2 * q * Q : 2 * q * Q + Q], in_=dst[:, 0:Q]
                )
                nc.vector.tensor_scalar(
                    dst[:, Q : 2 * Q],
                    cd[:, Q : 2 * Q],
                    b2sb,
                    0.0,
                    add_op,
                    max_op,
                )
                nc.sync.dma_start(
                    out=pout[:, 2 * q * Q + Q : 2 * (q + 1) * Q],
                    in_=dst[:, Q : 2 * Q],
                )
            else:
                nc.vector.tensor_scalar(
                    pair_view(dst, qc), pair_view(cd, qc), b2sb, 0.0, add_op, max_op
                )
                finish_quantum(q, dst, pc, h2r)

            xc = xc_n
            ab = ab_n

    nc.compile()
    return nc


def _get_program(c4: int, lastq: int = Q_COLS):
    key = (c4, lastq)
    if key not in _BUILD_CACHE:
        _BUILD_CACHE[key] = _build_program(c4, lastq)
    return _BUILD_CACHE[key]


def kernel(x, batch, num_graphs, W_in, b_in, W_h, b_h, W_out, b_out):
    from concourse import bass_utils

    x = np.asarray(x, dtype=np.float32)
    batch = np.asarray(batch).astype(np.int64)
    g_total = int(num_graphs)
    W_in = np.asarray(W_in, dtype=np.float32)
    b_in = np.asarray(b_in, dtype=np.float32)
    W_h = np.asarray(W_h, dtype=np.float32)
    b_h = np.asarray(b_h, dtype=np.float32)
    W_out = np.asarray(W_out, dtype=np.float32)
    b_out = np.asarray(b_out, dtype=np.float32)

    if batch.size and np.any(np.diff(batch) < 0):
        order = np.argsort(batch, kind="stable")
        x = x[order]
        batch = batch[order]

    n_nodes, f_in = x.shape
    h_dim = W_in.shape[1]
    assert f_in == F_IN and h_dim == H_DIM

    counts = np.bincount(batch, minlength=g_total).astype(np.int64)
    node_starts = np.concatenate([[0], np.cumsum(counts)])  # [G+1]

    # Per-graph padded counts (multiple of PACK).
    pc_counts = (counts + PACK - 1) // PACK * PACK

    # Balanced graph->core assignment (LPT): the SPMD program runs every core
    # for the max core's column count, so minimizing the max load saves whole
    # quanta versus contiguous 1024-graph shards (~0.6% spread).
    import heapq

    order_g = np.argsort(pc_counts, kind="stable")[::-1]
    heap = [(0, c) for c in range(N_CORES)]
    heapq.heapify(heap)
    core_graphs = [[] for _ in range(N_CORES)]
    loads = [0] * N_CORES
    for g in order_g:
        load, c = heapq.heappop(heap)
        core_graphs[c].append(g)
        load += int(pc_counts[g])
        loads[c] = load
        heapq.heappush(heap, (load, c))
    core_graphs = [np.asarray(gs, dtype=np.int64) for gs in core_graphs]
    c4_per_core = [t // PACK for t in loads]
    c_exact = max(c4_per_core)  # exact max columns across cores
    nq_h = (c_exact + Q_COLS - 1) // Q_COLS
    c4 = nq_h * Q_COLS
    lastq = (
        c_exact - (nq_h - 1) * Q_COLS
        if (RAGGED_LAST and c_exact > (nq_h - 1) * Q_COLS)
        else Q_COLS
    )

    # Constant tensors shared by all cores.
    w1blk = np.zeros((128, 128), dtype=np.float32)
    w1blk[0:32, 0:64] = W_in
    w1blk[32:64, 64:128] = W_in
    w1blk[64:96, 0:64] = W_in
    w1blk[96:128, 64:128] = W_in
    w1blk = w1blk.astype(ml_dtypes.bfloat16)
    w2blk = np.zeros((128, 128), dtype=np.float32)
    w2blk[0:64, 0:64] = W_h
    w2blk[64:128, 64:128] = W_h
    w2blk = w2blk.astype(ml_dtypes.bfloat16)
    b1cat = np.tile(b_in, 2).reshape(128, 1).astype(np.float32)
    b2cat = np.tile(b_h, 2).reshape(128, 1).astype(np.float32)

    # Per-core packed inputs.
    in_maps = []
    for c in range(N_CORES):
        gc = core_graphs[c]
        cnt_c = counts[gc]
        pc_c = pc_counts[gc]
        pad_starts = np.concatenate([[0], np.cumsum(pc_c)])  # [len(gc)+1]

        x_padded = np.zeros((c4 * PACK, f_in), dtype=np.float32)
        tot = int(cnt_c.sum())
        if tot > 0:
            # Gather this core's nodes graph-by-graph into padded slots.
            within = np.arange(tot) - np.repeat(
                np.concatenate([[0], np.cumsum(cnt_c)])[:-1], cnt_c
            )
            src = np.repeat(node_starts[gc], cnt_c) + within
            dst = np.repeat(pad_starts[:-1], cnt_c) + within
            x_padded[dst] = x[src]
        xT_dev = (
            x_padded.reshape(c4, PACK, f_in).transpose(1, 2, 0).reshape(128, c4)
        )
        xdt_np = ml_dtypes.float8_e4m3 if X_FP8 else ml_dtypes.bfloat16
        xT_dev = np.ascontiguousarray(xT_dev).astype(xdt_np)
        in_maps.append(
            {
                "xT": xT_dev,
                "w1": w1blk,
                "w2": w2blk,
                "b1": b1cat,
                "b2": b2cat,
            }
        )

    global _LAST_IN_MAPS
    _LAST_IN_MAPS = in_maps

    nc = _get_program(c4, lastq)
    res = bass_utils.run_bass_kernel_spmd(
        nc, in_maps, core_ids=list(range(N_CORES))
    )

    # Pad-node contribution, exactly as the device computes it for x=0 rows:
    # h1raw = 0 (fp32 psum) -> D1: bf16(relu(b1)) -> S2 bf16 matmul (fp32 acc)
    # -> D2: bf16(relu(. + b2)).
    bf = ml_dtypes.bfloat16
    h1p = np.maximum(b_in, 0.0).astype(bf).astype(np.float32)
    w2bf = W_h.astype(bf).astype(np.float32)
    vpad = np.maximum(h1p @ w2bf + b_h, 0.0).astype(bf).astype(np.float32)
    vpad_bf = vpad

    out = np.zeros((g_total, W_out.shape[1]), dtype=np.float32)
    for c in range(N_CORES):
        gc = core_graphs[c]
        cnt_c = counts[gc].astype(np.float64)
        pc_c = pc_counts[gc]
        pad_starts = np.concatenate([[0], np.cumsum(pc_c)])
        col_starts = pad_starts // PACK  # [len(gc)+1]

        nq = c4 // Q_COLS
        P = np.asarray(res.results[c]["pout"]).astype(np.float32)
        if DEVICE_R:
            R1 = P[0:64, :] + P[64:128, :]  # [64, c4]
        else:
            # fold partition halves (node pairs) and streams; the final
            # quantum may be ragged (lastq < Q_COLS real columns).
            nf = nq - 1
            Pf = P[:, : nf * 2 * Q_COLS].reshape(128, nf, 2, Q_COLS)
            R1f = (Pf[0:64] + Pf[64:128]).sum(axis=2).reshape(
                64, nf * Q_COLS
            )
            Pl = P[
                :, nf * 2 * Q_COLS : nf * 2 * Q_COLS + 2 * lastq
            ].reshape(128, 2, lastq)
            R1l = (Pl[0:64] + Pl[64:128]).sum(axis=1)
            R1 = np.concatenate([R1f, R1l], axis=1)  # [64, c_exact]
        cs = np.concatenate(
            [np.zeros((64, 1)), np.cumsum(R1.astype(np.float64), axis=1)], axis=1
        )  # [64, c4+1]
        seg_sum = (cs[:, col_starts[1:]] - cs[:, col_starts[:-1]]).T  # [g, 64]

        n_pad = (pc_c - counts[gc]).astype(np.float64)
        seg_sum = seg_sum - n_pad[:, None] * vpad_bf[None, :].astype(np.float64)
        denom = np.maximum(cnt_c, 1.0)
        mean = seg_sum / denom[:, None]
        mean[cnt_c == 0] = 0.0
        out[gc] = mean.astype(np.float32) @ W_out + b_out

    return out

